# revision 35
# baseline (speedup 1.0000x reference)
"""Trainium2 Bass kernel for one LLaMA transformer layer (TP over 8 cores).

Strategy (Megatron-style tensor parallel, feature-major on-chip layout):
- Activations live as x^T [D, S] tiles (partition = d_model chunk, free = seq).
- Each core owns H/8 heads of q/k/v/wo and 1/8 of (padded) d_ff.
- RMSNorm: sumsq via square + ones-matmul; rstd via one ACT Rsqrt; the 1/rms
  column scale is folded into PSUM evictions (commutes with the contraction
  over D).
- RoPE: head-dim pairs pre-permuted into [even|odd] halves inside each head's
  weight columns; rotation = two full-tile multiplies plus a half-swap DMA.
- Attention: scores transposed (attnT[sk, sq]); exp(s/sqrt(dh) - 10);
  ones-matmul denominator; ACT-reciprocal; causal masking via precomputed
  0/1 tiles on diagonal blocks.
- Collectives per seq block on gpsimd, overlapped with compute.

This revision is built around keeping the PE warm (HAM K=8/8) and the DMA
trigger queues quiet:
- x block resident in SBUF (one [P,32,SB] tile reused attn->ffn as x_mid).
- All weight streams batched into multi-chunk DMAs via DRAM AP rearrange
  (tens of DMA triggers per block instead of ~900).
- Big streaming loads ride the scalar (ACT) HWDGE queue; small
  latency-critical transfers ride the sync (SP) queue; collectives gpsimd.
- Single uniform [P,SB] fp32 PSUM slot scheme: "acc" bufs=6 (long-lived
  accumulators) + "sc" bufs=2 (score tiles) = exactly 8 banks.
- Evictions prefer DVE (tensor_copy/tensor_tensor, bf16 2x rate); ACT does
  exp/sigmoid/rsqrt/reciprocal/per-partition scales only.
"""

import sys

for _p in ("/opt/trn_rl_repo", "/root/.axon_site/_ro/trn_rl_repo"):
    if _p not in sys.path:
        sys.path.append(_p)

import contextlib

import numpy as np
import ml_dtypes

import concourse.bass as bass  # noqa: F401
import concourse.tile as tile
from concourse import bacc, mybir
from concourse.bass_utils import run_bass_kernel_spmd

F32 = mybir.dt.float32
BF16 = mybir.dt.bfloat16
AF = mybir.ActivationFunctionType
P = 128


class Cfg:
    def __init__(self, S, SB, D, H, DH, DFF_PAD, n_cores=8):
        self.S, self.SB, self.D, self.H, self.DH = S, SB, D, H, DH
        self.n_cores = n_cores
        self.NB = S // SB            # seq blocks
        self.KD = D // P             # d_model chunks
        self.HL = H // n_cores       # heads per core
        self.FL = self.HL * DH       # local qkv features
        self.FC = self.FL // P       # local feature chunks (== HL for DH=128)
        self.DFF_PAD = DFF_PAD
        self.DFL = DFF_PAD // n_cores  # local (padded) d_ff
        self.DFC = self.DFL // P
        self.JT = SB // P            # sk-tiles per seq block
        self.DS = D // n_cores       # d_model slice per core (output rows)
        assert S % SB == 0 and SB % P == 0 and D % P == 0
        assert H % n_cores == 0 and DH == P and self.DFL % P == 0
        assert self.DS % P == 0


CFG_FULL = Cfg(S=2048, SB=512, D=4096, H=32, DH=128, DFF_PAD=11264)


def build_program(c: Cfg):
    nc = bacc.Bacc("TRN2", target_bir_lowering=False, debug=False,
                   num_devices=c.n_cores)

    def din(name, shape, dt):
        return nc.dram_tensor(name, list(shape), dt, kind="ExternalInput").ap()

    xT16 = din("xT16", (c.D, c.S), BF16)
    xTf32s = din("xTf32s", (c.DS, c.S), F32)
    wq = din("wq_s", (c.D, c.FL), BF16)
    wk = din("wk_s", (c.D, c.FL), BF16)
    wv = din("wv_s", (c.D, c.FL), BF16)
    wo = din("wo_s", (c.FL, c.D), BF16)
    w1g = din("w1g_s", (c.D, 2 * c.DFL), BF16)
    w2 = din("w2_s", (c.DFL, c.D), BF16)
    cosd = din("cosd16", (P, c.S), BF16)
    sinpm = din("sinpm16", (P, c.S), BF16)
    caus = din("caus", (c.JT, P, c.SB), BF16)
    swp = din("swp", (P, P), BF16)
    outT = nc.dram_tensor("outT", [c.DS, c.S], F32, kind="ExternalOutput").ap()

    # 3D views of the weight matrices: [P, k-chunk, cols]
    xT3 = xT16.rearrange("(o p) s -> p o s", p=P)
    wq3 = wq.rearrange("(o p) f -> p o f", p=P)
    wk3 = wk.rearrange("(o p) f -> p o f", p=P)
    wv3 = wv.rearrange("(o p) f -> p o f", p=P)
    wo3 = wo.rearrange("(o p) d -> p o d", p=P)
    w1g3 = w1g.rearrange("(o p) f -> p o f", p=P)
    w23 = w2.rearrange("(o p) d -> p o d", p=P)
    xf3 = xTf32s.rearrange("(o p) s -> p o s", p=P)
    out3 = outT.rearrange("(o p) s -> p o s", p=P)

    rg = [list(range(c.n_cores))]
    inv_sqrt_dh = 1.0 / float(np.sqrt(c.DH))
    KD, FC, JT, SB, DFC = c.KD, c.FC, c.JT, c.SB, c.DFC

    with tile.TileContext(nc) as tc:
        ctx = contextlib.ExitStack()
        with ctx:
            cons = ctx.enter_context(tc.tile_pool(name="cons", bufs=1))
            sb = ctx.enter_context(tc.tile_pool(name="sb", bufs=1))
            ps = ctx.enter_context(tc.tile_pool(name="ps", bufs=1, space="PSUM"))
            dram = ctx.enter_context(tc.tile_pool(name="dram", bufs=1,
                                                  space="DRAM"))

            # ---- constants ----
            ones16 = cons.tile([P, P], BF16)
            nc.vector.memset(ones16, 1.0)
            eps_b = cons.tile([P, 1], F32)
            nc.vector.memset(eps_b, 1e-6)
            neg10_b = cons.tile([P, 1], F32)
            nc.vector.memset(neg10_b, -10.0)
            zero_b = cons.tile([P, 1], F32)
            nc.vector.memset(zero_b, 0.0)
            caus_sb = cons.tile([P, JT, SB], BF16)
            nc.sync.dma_start(out=caus_sb,
                              in_=caus.rearrange("j p f -> p j f"))
            swp_sb = cons.tile([P, P], BF16)
            nc.sync.dma_start(out=swp_sb, in_=swp)

            # ---- DRAM bounce buffers for collectives ----
            d1c, d1r, d1g, d2c, d2r = [], [], [], [], []
            for b in range(c.NB):
                d1c.append(dram.tile([c.D, SB], BF16, name=f"d1c{b}"))
                d1r.append(dram.tile([c.DS, SB], BF16, name=f"d1r{b}"))
                d1g.append(dram.tile([c.D, SB], BF16, name=f"d1g{b}",
                                     addr_space="Shared"))
                d2c.append(dram.tile([c.D, SB], BF16, name=f"d2c{b}"))
                d2r.append(dram.tile([c.DS, SB], BF16, name=f"d2r{b}"))

            # persistent per-core activations (whole attention half)
            kT3 = cons.tile([P, c.HL, c.S], BF16)      # k^T feat-major
            v3 = cons.tile([P, c.S // P, c.FL], BF16)  # v seq-major

            def acc():
                return ps.tile([P, SB], F32, name="acc", bufs=6)

            def sc_ps():
                return ps.tile([P, SB], F32, name="sc", bufs=2)

            def load_x_block(b):
                """x(b) -> resident [P, KD, SB] tile (4 x 1MB DMAs)."""
                t = sb.tile([P, KD, SB], BF16, name="xm", bufs=1)
                sl = slice(b * SB, (b + 1) * SB)
                for g in range(0, KD, 8):
                    nc.sync.dma_start(out=t[:, g:g + 8, :],
                                      in_=xT3[:, g:g + 8, sl])
                return t

            def rmsnorm_stats(xm, name):
                """sumsq -> rstd [P,SB] f32 (per-token, bcast over parts)."""
                ss = acc()
                for k in range(KD):
                    x2 = sb.tile([P, SB], BF16, name="x2", bufs=2)
                    nc.vector.tensor_mul(x2, xm[:, k, :], xm[:, k, :])
                    nc.tensor.matmul(ss, ones16, x2,
                                     start=(k == 0), stop=(k == KD - 1))
                rms = sb.tile([P, SB], F32, name="rms", bufs=2)
                nc.scalar.activation(rms, ss, AF.Sqrt, bias=eps_b,
                                     scale=1.0 / c.D)
                rstd = sb.tile([P, SB], F32, name=name, bufs=2)
                nc.vector.reciprocal_approx_fast(rstd, rms)
                return rstd

            xm = load_x_block(0)

            # Deferred collective state: (block, held d1t tile) whose last
            # out-proj store group is released only after the NEXT block's
            # QKV passes — a true data dependency that shifts the RS into
            # the SBUF-only attention window, keeping HBM quiet while the
            # weight slabs for the following block stream in.
            pend = {"att": None, "ffn": None}

            def release_attn_rs(dep_tile):
                bb, held = pend["att"]
                out_ap = d1c[bb][(KD - 4) * P:KD * P, :].rearrange(
                    "(o p) f -> p o f", p=P)
                rel = sb.tile([P, 4, SB], BF16, name="d1rel", bufs=1)
                for i in range(4):
                    nc.vector.scalar_tensor_tensor(
                        rel[:, i, :], dep_tile, 0.0, held[:, i, :],
                        mybir.AluOpType.mult, mybir.AluOpType.add)
                nc.sync.dma_start(out=out_ap, in_=rel)
                nc.gpsimd.collective_compute(
                    "ReduceScatter", mybir.AluOpType.add, replica_groups=rg,
                    ins=[d1c[bb][:]], outs=[d1r[bb][:]])
                nc.gpsimd.collective_compute(
                    "AllGather", mybir.AluOpType.bypass, replica_groups=rg,
                    ins=[d1r[bb][:]], outs=[d1g[bb][:]])
                pend["att"] = None

            # =========== attention half ===========
            for b in range(c.NB):
                sl = slice(b * SB, (b + 1) * SB)

                rstd = rmsnorm_stats(xm, "rstd")
                # per-seq-tile [P,1] transposed copies (for v's ACT scale)
                rstdT = sb.tile([P, JT], F32, name="rstdT", bufs=2)
                for st in range(JT):
                    nc.sync.dma_start(out=rstdT[:, st:st + 1],
                                      in_=rstd[0:1, st * P:(st + 1) * P])
                # rope tables pre-scaled by rstd for this block
                cosR = sb.tile([P, SB], BF16, name="cosR", bufs=2)
                nc.sync.dma_start(out=cosR, in_=cosd[:, sl])
                nc.vector.tensor_mul(cosR, cosR, rstd)
                sinR = sb.tile([P, SB], BF16, name="sinR", bufs=2)
                nc.sync.dma_start(out=sinR, in_=sinpm[:, sl])
                nc.vector.tensor_mul(sinR, sinR, rstd)

                qT_blk = sb.tile([P, c.HL, SB], BF16, name="qT_blk", bufs=1)

                def rope_evict(psl, dst, dsl):
                    # psl: 4 psum accumulators; dst[_, h, dsl] targets.
                    # Half-swap via a PE permutation matmul (no DMA on the
                    # critical path — collective traffic can't stall it).
                    for pr in range(FC // 2):
                        raw = sb.tile([P, 2, SB], BF16, name="rp_raw", bufs=2)
                        nc.vector.tensor_copy(raw[:, 0, :], psl[2 * pr])
                        nc.vector.tensor_copy(raw[:, 1, :], psl[2 * pr + 1])
                        sw_ps = [sc_ps(), sc_ps()]
                        nc.tensor.matmul(sw_ps[0], swp_sb, raw[:, 0, :],
                                         start=True, stop=True)
                        nc.tensor.matmul(sw_ps[1], swp_sb, raw[:, 1, :],
                                         start=True, stop=True)
                        cb = cosR[:, None, :].to_broadcast([P, 2, SB])
                        nc.vector.tensor_mul(raw, raw, cb)
                        for i in range(2):
                            swm = sb.tile([P, SB], BF16, name="swm", bufs=2)
                            nc.vector.tensor_mul(swm, sw_ps[i], sinR)
                            nc.vector.tensor_add(dst[:, 2 * pr + i, dsl],
                                                 raw[:, i, :], swm)

                def qkv_pass(w3, n):
                    psl = [acc() for _ in range(FC)]
                    for g in range(0, KD, 4):
                        slab = sb.tile([P, 4, c.FL], BF16, name="wqkv_sl",
                                       bufs=3)
                        nc.scalar.dma_start(out=slab, in_=w3[:, g:g + 4, :])
                        for kk in range(4):
                            for h in range(FC):
                                nc.tensor.matmul(
                                    psl[h], slab[:, kk, h * P:(h + 1) * P],
                                    xm[:, g + kk, :], start=(g + kk == 0),
                                    stop=(g + kk == KD - 1))
                    return psl

                psq = qkv_pass(wq3, "wq_sl")
                rope_evict(psq, qT_blk, slice(0, SB))
                psk = qkv_pass(wk3, "wk_sl")
                rope_evict(psk, kT3, sl)

                # v: seq-major, rstd applied as per-partition ACT scale
                psv = [acc() for _ in range(JT)]
                for g in range(0, KD, 4):
                    slab = sb.tile([P, 4, c.FL], BF16, name="wqkv_sl", bufs=3)
                    nc.scalar.dma_start(out=slab, in_=wv3[:, g:g + 4, :])
                    for kk in range(4):
                        for st in range(JT):
                            nc.tensor.matmul(
                                psv[st], xm[:, g + kk, st * P:(st + 1) * P],
                                slab[:, kk, :], start=(g + kk == 0),
                                stop=(g + kk == KD - 1))
                for st in range(JT):
                    nc.scalar.activation(v3[:, b * JT + st, :], psv[st],
                                         AF.Copy, scale=rstdT[:, st:st + 1])

                # QKV slabs are in: release the previous block's collectives
                if pend["att"] is not None:
                    release_attn_rs(v3[:, b * JT + JT - 1, :])

                # prefetch next block's x while attention runs
                if b + 1 < c.NB:
                    xm_next = load_x_block(b + 1)
                else:
                    xm_next = None

                # -- phase 3: attention, two heads in flight --
                oT_blk = sb.tile([P, c.HL, SB], BF16, name="oT_blk", bufs=1)
                nt = (b + 1) * JT  # valid sk tiles
                for hp in range(c.HL // 2):
                    hs = (2 * hp, 2 * hp + 1)
                    den = {h: acc() for h in hs}
                    o_ps = {h: acc() for h in hs}
                    for t in range(nt):
                        j = t - b * JT
                        for h in hs:
                            scp = sc_ps()
                            nc.tensor.matmul(scp, kT3[:, h, t * P:(t + 1) * P],
                                             qT_blk[:, h, :], start=True,
                                             stop=True)
                            at = sb.tile([P, SB], BF16, name="at", bufs=3)
                            nc.scalar.activation(at, scp, AF.Exp, bias=neg10_b,
                                                 scale=inv_sqrt_dh)
                            if j >= 0:  # diagonal tile: apply causal mask
                                atm = sb.tile([P, SB], BF16, name="atm",
                                              bufs=2)
                                nc.vector.tensor_mul(atm, at, caus_sb[:, j, :])
                                at = atm
                            nc.tensor.matmul(den[h], ones16, at,
                                             start=(t == 0), stop=(t == nt - 1))
                            nc.tensor.matmul(o_ps[h],
                                             v3[:, t, h * P:(h + 1) * P], at,
                                             start=(t == 0), stop=(t == nt - 1))
                    for h in hs:
                        rden = sb.tile([P, SB], F32, name="rden", bufs=1)
                        nc.vector.reciprocal_approx_fast(rden, den[h])
                        nc.vector.tensor_mul(oT_blk[:, h, :], o_ps[h], rden)

                # -- phase 4: out-projection (4 md chunks per group) --
                # Last group's store is held back; releasing it after the
                # next block's QKV passes is what defers the collective.
                for mg in range(0, KD, 4):
                    slab = sb.tile([P, FC, 4 * P], BF16, name="wo_sl", bufs=2)
                    nc.sync.dma_start(out=slab,
                                      in_=wo3[:, :, mg * P:(mg + 4) * P])
                    pso = [acc() for _ in range(4)]
                    for f in range(FC):
                        for i in range(4):
                            nc.tensor.matmul(pso[i],
                                             slab[:, f, i * P:(i + 1) * P],
                                             oT_blk[:, f, :], start=(f == 0),
                                             stop=(f == FC - 1))
                    last = mg == KD - 4
                    d1t = sb.tile([P, 4, SB], BF16,
                                  name="d1hold" if last else "d1t",
                                  bufs=1 if last else 2)
                    for i in range(4):
                        nc.vector.tensor_copy(d1t[:, i, :], pso[i])
                    if last:
                        pend["att"] = (b, d1t)
                    else:
                        nc.sync.dma_start(
                            out=d1c[b][mg * P:(mg + 4) * P, :].rearrange(
                                "(o p) f -> p o f", p=P),
                            in_=d1t)

                xm = xm_next

            # =========== FFN half ===========
            def phase8(b):
                """final residual for this core's d_model slice, block b.
                Emitted one block late so the RS(b) wait is already met."""
                sl = slice(b * SB, (b + 1) * SB)
                d1r3 = d1r[b].rearrange("(o p) f -> p o f", p=P)
                d2r3 = d2r[b].rearrange("(o p) f -> p o f", p=P)
                for g in range(0, c.DS // P, 2):
                    gs_ = slice(g, g + 2)
                    xf = sb.tile([P, 2, SB], F32, name="xf", bufs=1)
                    nc.sync.dma_start(out=xf, in_=xf3[:, gs_, sl])
                    r1 = sb.tile([P, 2, SB], BF16, name="r1", bufs=1)
                    nc.sync.dma_start(out=r1, in_=d1r3[:, gs_, :])
                    r2 = sb.tile([P, 2, SB], BF16, name="r2", bufs=1)
                    nc.sync.dma_start(out=r2, in_=d2r3[:, gs_, :])
                    r12 = sb.tile([P, 2, SB], F32, name="r12", bufs=1)
                    nc.vector.tensor_add(r12, r1, r2)
                    nc.vector.tensor_add(xf, xf, r12)
                    nc.sync.dma_start(out=out3[:, gs_, sl], in_=xf)

            def load_xmid(b):
                """phase 5: x_mid = x + d1 via gpsimd accumulate-DMA."""
                t = load_x_block(b)
                d1g3 = d1g[b].rearrange("(o p) f -> p o f", p=P)
                for g in range(0, KD, 4):
                    nc.gpsimd.dma_start(out=t[:, g:g + 4, :],
                                        in_=d1g3[:, g:g + 4, :],
                                        accum_op=mybir.AluOpType.add)
                return t

            def release_ffn_rs(dep_tile):
                bb, held = pend["ffn"]
                out_ap = d2c[bb][(KD - 2) * P:KD * P, :].rearrange(
                    "(o p) f -> p o f", p=P)
                if dep_tile is None:
                    nc.sync.dma_start(out=out_ap, in_=held)
                else:
                    rel = sb.tile([P, 2, SB], BF16, name="d2rel", bufs=1)
                    for i in range(2):
                        nc.vector.scalar_tensor_tensor(
                            rel[:, i, :], dep_tile, 0.0, held[:, i, :],
                            mybir.AluOpType.mult, mybir.AluOpType.add)
                    nc.sync.dma_start(out=out_ap, in_=rel)
                nc.gpsimd.collective_compute(
                    "ReduceScatter", mybir.AluOpType.add, replica_groups=rg,
                    ins=[d2c[bb][:]], outs=[d2r[bb][:]])
                pend["ffn"] = None

            xm = load_xmid(0)
            for b in range(c.NB):
                sl = slice(b * SB, (b + 1) * SB)
                rstd2 = rmsnorm_stats(xm, "rstd2")

                # -- phase 6: FFN-A: h = (xn@w1) * silu(xn@wg) --
                h_blk = sb.tile([P, DFC, SB], BF16, name="h_blk", bufs=1)
                for j in range(DFC):
                    pa = acc()
                    pg = acc()
                    for g in range(0, KD, 8):
                        slab = sb.tile([P, 8, 2 * P], BF16, name="w1g_sl",
                                       bufs=3)
                        nc.scalar.dma_start(
                            out=slab,
                            in_=w1g3[:, g:g + 8, j * 2 * P:(j + 1) * 2 * P])
                        for kk in range(8):
                            nc.tensor.matmul(pa, slab[:, kk, 0:P],
                                             xm[:, g + kk, :],
                                             start=(g + kk == 0),
                                             stop=(g + kk == KD - 1))
                            nc.tensor.matmul(pg, slab[:, kk, P:2 * P],
                                             xm[:, g + kk, :],
                                             start=(g + kk == 0),
                                             stop=(g + kk == KD - 1))
                    a_t = sb.tile([P, SB], BF16, name="a_t", bufs=2)
                    nc.vector.tensor_mul(a_t, pa, rstd2)
                    g_t = sb.tile([P, SB], BF16, name="g_t", bufs=2)
                    nc.vector.tensor_mul(g_t, pg, rstd2)
                    gs = sb.tile([P, SB], BF16, name="gs", bufs=2)
                    nc.scalar.activation(gs, g_t, AF.Sigmoid, bias=zero_b)
                    nc.vector.tensor_mul(a_t, a_t, g_t)
                    nc.vector.tensor_mul(h_blk[:, j, :], a_t, gs)

                    if j == 2:
                        # early FFN-A slabs are in: release pending RS's
                        if pend["att"] is not None:
                            release_attn_rs(h_blk[:, 2, :])
                        if pend["ffn"] is not None:
                            release_ffn_rs(h_blk[:, 2, :])

                # prefetch next block's x_mid during FFN-B
                xm = load_xmid(b + 1) if b + 1 < c.NB else None

                if b > 0:
                    phase8(b - 1)

                # -- phase 7: FFN-B: d2 = h @ w2 (2 md chunks per group) --
                for mg in range(0, KD, 2):
                    slab = sb.tile([P, DFC, 2 * P], BF16, name="w2_sl", bufs=2)
                    nc.scalar.dma_start(out=slab,
                                        in_=w23[:, :, mg * P:(mg + 2) * P])
                    ps2 = [acc() for _ in range(2)]
                    for j in range(DFC):
                        for i in range(2):
                            nc.tensor.matmul(ps2[i],
                                             slab[:, j, i * P:(i + 1) * P],
                                             h_blk[:, j, :], start=(j == 0),
                                             stop=(j == DFC - 1))
                    last = mg == KD - 2
                    d2t = sb.tile([P, 2, SB], BF16,
                                  name="d2hold" if last else "d2t",
                                  bufs=1 if last else 2)
                    for i in range(2):
                        nc.vector.tensor_copy(d2t[:, i, :], ps2[i])
                    if last:
                        pend["ffn"] = (b, d2t)
                    else:
                        nc.sync.dma_start(
                            out=d2c[b][mg * P:(mg + 2) * P, :].rearrange(
                                "(o p) f -> p o f", p=P),
                            in_=d2t)

            release_ffn_rs(None)
            phase8(c.NB - 1)

    nc.compile()
    return nc


# ---------------- host-side data prep ----------------

def _bf16(a):
    return np.asarray(a, np.float32).astype(ml_dtypes.bfloat16)


def prep_in_maps(c: Cfg, x, wq, bq, wk, bk, wv, bv, wo, bo, scale1, scale2,
                 w1, b1, wg, bg, w2, b2):
    x = np.asarray(x, np.float32).reshape(c.S, c.D)
    for name, bias in (("bq", bq), ("bk", bk), ("bv", bv), ("bo", bo),
                       ("b1", b1), ("bg", bg), ("b2", b2)):
        assert not np.any(np.asarray(bias)), f"{name} must be zero"

    scale1 = np.asarray(scale1, np.float32)
    scale2 = np.asarray(scale2, np.float32)
    wq = np.asarray(wq, np.float32) * scale1[:, None]
    wk = np.asarray(wk, np.float32) * scale1[:, None]
    wv = np.asarray(wv, np.float32) * scale1[:, None]
    wo = np.asarray(wo, np.float32)
    w1 = np.asarray(w1, np.float32) * scale2[:, None]
    wg = np.asarray(wg, np.float32) * scale2[:, None]
    w2 = np.asarray(w2, np.float32)

    # rope permutation within each head's DH columns: [evens | odds]
    perm = np.concatenate([np.arange(0, c.DH, 2), np.arange(1, c.DH, 2)])
    fullperm = np.concatenate([h * c.DH + perm for h in range(c.H)])
    wq_p = wq[:, fullperm]
    wk_p = wk[:, fullperm]

    # pad FFN to DFF_PAD columns/rows
    dff = w1.shape[1]
    pad = c.DFF_PAD - dff
    w1p = np.pad(w1, ((0, 0), (0, pad)))
    wgp = np.pad(wg, ((0, 0), (0, pad)))
    w2p = np.pad(w2, ((0, pad), (0, 0)))

    # rope tables (feat-major, duplicated cos / +-sin halves)
    theta = 1.0 / (10000.0 ** (np.arange(0, c.DH, 2, dtype=np.float32) / c.DH))
    pos = np.arange(c.S, dtype=np.float32)
    ang = pos[None, :] * theta[:, None]            # [DH/2, S]
    cosd = np.concatenate([np.cos(ang), np.cos(ang)], 0).astype(np.float32)
    sinpm = np.concatenate([-np.sin(ang), np.sin(ang)], 0).astype(np.float32)
    assert cosd.shape == (P, c.S)

    # causal masks for diagonal tiles
    caus = np.zeros((c.JT, P, c.SB), np.float32)
    f = np.arange(c.SB)
    p_ = np.arange(P)
    for j in range(c.JT):
        caus[j] = (f[None, :] >= (P * j + p_[:, None])).astype(np.float32)
    caus = caus.astype(ml_dtypes.bfloat16)

    xT = np.ascontiguousarray(x.T)                 # [D, S]
    xT16 = _bf16(xT)
    swp = np.roll(np.eye(P, dtype=np.float32), P // 2, axis=1)

    in_maps = []
    for core in range(c.n_cores):
        fs = slice(core * c.FL, (core + 1) * c.FL)
        dsl = slice(core * c.DFL, (core + 1) * c.DFL)
        ms = slice(core * c.DS, (core + 1) * c.DS)
        w1c = w1p[:, dsl]
        wgc = wgp[:, dsl]
        # interleave per 128-col chunk: [w1 j | wg j] pairs
        w1g = np.empty((c.D, 2 * c.DFL), np.float32)
        for j in range(c.DFC):
            w1g[:, j * 2 * P:j * 2 * P + P] = w1c[:, j * P:(j + 1) * P]
            w1g[:, j * 2 * P + P:(j + 1) * 2 * P] = wgc[:, j * P:(j + 1) * P]
        in_maps.append({
            "xT16": xT16,
            "xTf32s": np.ascontiguousarray(xT[ms]),
            "wq_s": _bf16(wq_p[:, fs]),
            "wk_s": _bf16(wk_p[:, fs]),
            "wv_s": _bf16(wv[:, fs]),
            "wo_s": _bf16(wo[fs, :]),
            "w1g_s": _bf16(w1g),
            "w2_s": _bf16(w2p[dsl, :]),
            "cosd16": _bf16(cosd),
            "sinpm16": _bf16(sinpm),
            "caus": caus,
            "swp": _bf16(swp),
        })
    return in_maps


def assemble_output(c: Cfg, results):
    outT = np.concatenate([results[core]["outT"] for core in range(c.n_cores)],
                          axis=0)               # [D, S]
    return np.ascontiguousarray(outT.T).reshape(1, c.S, c.D)


_CACHED = {}


def kernel(**inputs) -> np.ndarray:
    c = CFG_FULL
    if "nc" not in _CACHED:
        _CACHED["nc"] = build_program(c)
    nc = _CACHED["nc"]
    in_maps = prep_in_maps(c, **{k: np.asarray(inputs[k]) for k in (
        "x", "wq", "bq", "wk", "bk", "wv", "bv", "wo", "bo",
        "scale1", "scale2", "w1", "b1", "wg", "bg", "w2", "b2")})
    res = run_bass_kernel_spmd(nc, in_maps, core_ids=list(range(c.n_cores)))
    return assemble_output(c, res.results)


# revision 41
# speedup vs baseline: 1.0109x; 1.0109x over previous
"""Trainium2 Bass kernel for one LLaMA transformer layer (TP over 8 cores).

Strategy (Megatron-style tensor parallel, feature-major on-chip layout):
- Activations live as x^T [D, S] tiles (partition = d_model chunk, free = seq).
- Each core owns H/8 heads of q/k/v/wo and 1/8 of (padded) d_ff.
- RMSNorm: sumsq via square + ones-matmul; rstd via one ACT Rsqrt; the 1/rms
  column scale is folded into PSUM evictions (commutes with the contraction
  over D).
- RoPE: head-dim pairs pre-permuted into [even|odd] halves inside each head's
  weight columns; rotation = two full-tile multiplies plus a half-swap DMA.
- Attention: scores transposed (attnT[sk, sq]); exp(s/sqrt(dh) - 10);
  ones-matmul denominator; ACT-reciprocal; causal masking via precomputed
  0/1 tiles on diagonal blocks.
- Collectives per seq block on gpsimd, overlapped with compute.

This revision is built around keeping the PE warm (HAM K=8/8) and the DMA
trigger queues quiet:
- x block resident in SBUF (one [P,32,SB] tile reused attn->ffn as x_mid).
- All weight streams batched into multi-chunk DMAs via DRAM AP rearrange
  (tens of DMA triggers per block instead of ~900).
- Big streaming loads ride the scalar (ACT) HWDGE queue; small
  latency-critical transfers ride the sync (SP) queue; collectives gpsimd.
- Single uniform [P,SB] fp32 PSUM slot scheme: "acc" bufs=6 (long-lived
  accumulators) + "sc" bufs=2 (score tiles) = exactly 8 banks.
- Evictions prefer DVE (tensor_copy/tensor_tensor, bf16 2x rate); ACT does
  exp/sigmoid/rsqrt/reciprocal/per-partition scales only.
"""

import sys

for _p in ("/opt/trn_rl_repo", "/root/.axon_site/_ro/trn_rl_repo"):
    if _p not in sys.path:
        sys.path.append(_p)

import contextlib

import numpy as np
import ml_dtypes

import concourse.bass as bass  # noqa: F401
import concourse.tile as tile
from concourse import bacc, mybir
from concourse.bass_utils import run_bass_kernel_spmd

F32 = mybir.dt.float32
BF16 = mybir.dt.bfloat16
AF = mybir.ActivationFunctionType
P = 128


class Cfg:
    def __init__(self, S, SB, D, H, DH, DFF_PAD, n_cores=8):
        self.S, self.SB, self.D, self.H, self.DH = S, SB, D, H, DH
        self.n_cores = n_cores
        self.NB = S // SB            # seq blocks
        self.KD = D // P             # d_model chunks
        self.HL = H // n_cores       # heads per core
        self.FL = self.HL * DH       # local qkv features
        self.FC = self.FL // P       # local feature chunks (== HL for DH=128)
        self.DFF_PAD = DFF_PAD
        self.DFL = DFF_PAD // n_cores  # local (padded) d_ff
        self.DFC = self.DFL // P
        self.JT = SB // P            # sk-tiles per seq block
        self.DS = D // n_cores       # d_model slice per core (output rows)
        assert S % SB == 0 and SB % P == 0 and D % P == 0
        assert H % n_cores == 0 and DH == P and self.DFL % P == 0
        assert self.DS % P == 0


CFG_FULL = Cfg(S=2048, SB=512, D=4096, H=32, DH=128, DFF_PAD=11264)


def build_program(c: Cfg):
    nc = bacc.Bacc("TRN2", target_bir_lowering=False, debug=False,
                   num_devices=c.n_cores)

    def din(name, shape, dt):
        return nc.dram_tensor(name, list(shape), dt, kind="ExternalInput").ap()

    xT16 = din("xT16", (c.D, c.S), BF16)
    xTf32s = din("xTf32s", (c.DS, c.S), F32)
    wq = din("wq_s", (c.D, c.FL), BF16)
    wk = din("wk_s", (c.D, c.FL), BF16)
    wv = din("wv_s", (c.D, c.FL), BF16)
    wo = din("wo_s", (c.FL, c.D), BF16)
    w1g = din("w1g_s", (c.D, 2 * c.DFL), BF16)
    w2 = din("w2_s", (c.DFL, c.D), BF16)
    cosd = din("cosd16", (P, c.S), BF16)
    sinpm = din("sinpm16", (P, c.S), BF16)
    caus = din("caus", (c.JT, P, c.SB), BF16)
    swp = din("swp", (P, P), BF16)
    outT = nc.dram_tensor("outT", [c.DS, c.S], F32, kind="ExternalOutput").ap()

    # 3D views of the weight matrices: [P, k-chunk, cols]
    xT3 = xT16.rearrange("(o p) s -> p o s", p=P)
    wq3 = wq.rearrange("(o p) f -> p o f", p=P)
    wk3 = wk.rearrange("(o p) f -> p o f", p=P)
    wv3 = wv.rearrange("(o p) f -> p o f", p=P)
    wo3 = wo.rearrange("(o p) d -> p o d", p=P)
    w1g3 = w1g.rearrange("(o p) f -> p o f", p=P)
    w23 = w2.rearrange("(o p) d -> p o d", p=P)
    xf3 = xTf32s.rearrange("(o p) s -> p o s", p=P)
    out3 = outT.rearrange("(o p) s -> p o s", p=P)

    rg = [list(range(c.n_cores))]
    inv_sqrt_dh = 1.0 / float(np.sqrt(c.DH))
    KD, FC, JT, SB, DFC = c.KD, c.FC, c.JT, c.SB, c.DFC

    with tile.TileContext(nc) as tc:
        ctx = contextlib.ExitStack()
        with ctx:
            cons = ctx.enter_context(tc.tile_pool(name="cons", bufs=1))
            sb = ctx.enter_context(tc.tile_pool(name="sb", bufs=1))
            ps = ctx.enter_context(tc.tile_pool(name="ps", bufs=1, space="PSUM"))
            dram = ctx.enter_context(tc.tile_pool(name="dram", bufs=1,
                                                  space="DRAM"))

            # ---- constants ----
            ones16 = cons.tile([P, P], BF16)
            nc.vector.memset(ones16, 1.0)
            eps_b = cons.tile([P, 1], F32)
            nc.vector.memset(eps_b, 1e-6)
            neg10_b = cons.tile([P, 1], F32)
            nc.vector.memset(neg10_b, -10.0)
            zero_b = cons.tile([P, 1], F32)
            nc.vector.memset(zero_b, 0.0)
            caus_sb = cons.tile([P, JT, SB], BF16)
            nc.sync.dma_start(out=caus_sb,
                              in_=caus.rearrange("j p f -> p j f"))
            swp_sb = cons.tile([P, P], BF16)
            nc.sync.dma_start(out=swp_sb, in_=swp)

            # ---- DRAM bounce buffers for collectives ----
            d1c, d1r, d1g, d2c, d2r = [], [], [], [], []
            for b in range(c.NB):
                d1c.append(dram.tile([c.D, SB], BF16, name=f"d1c{b}"))
                d1r.append(dram.tile([c.DS, SB], BF16, name=f"d1r{b}"))
                d1g.append(dram.tile([c.D, SB], BF16, name=f"d1g{b}",
                                     addr_space="Shared"))
                d2c.append(dram.tile([c.D, SB], BF16, name=f"d2c{b}"))
                d2r.append(dram.tile([c.DS, SB], BF16, name=f"d2r{b}"))

            # persistent per-core activations (whole attention half)
            kT3 = cons.tile([P, c.HL, c.S], BF16)      # k^T feat-major
            v3 = cons.tile([P, c.S // P, c.FL], BF16)  # v seq-major

            def acc():
                return ps.tile([P, SB], F32, name="acc", bufs=6)

            def sc_ps():
                return ps.tile([P, SB], F32, name="sc", bufs=2)

            def load_x_block(b):
                """x(b) -> resident [P, KD, SB] tile (4 x 1MB DMAs)."""
                t = sb.tile([P, KD, SB], BF16, name="xm", bufs=1)
                sl = slice(b * SB, (b + 1) * SB)
                for g in range(0, KD, 8):
                    nc.sync.dma_start(out=t[:, g:g + 8, :],
                                      in_=xT3[:, g:g + 8, sl])
                return t

            def rmsnorm_stats(xm, name):
                """sumsq -> rstd [P,SB] f32 (per-token, bcast over parts)."""
                ss = acc()
                for k in range(KD):
                    x2 = sb.tile([P, SB], BF16, name="x2", bufs=2)
                    nc.vector.tensor_mul(x2, xm[:, k, :], xm[:, k, :])
                    nc.tensor.matmul(ss, ones16, x2,
                                     start=(k == 0), stop=(k == KD - 1))
                rms = sb.tile([P, SB], F32, name="rms", bufs=2)
                nc.scalar.activation(rms, ss, AF.Sqrt, bias=eps_b,
                                     scale=1.0 / c.D)
                rstd = sb.tile([P, SB], F32, name=name, bufs=2)
                nc.vector.reciprocal_approx_fast(rstd, rms)
                return rstd

            xm = load_x_block(0)

            # Deferred collective state: (block, held d1t tile) whose last
            # out-proj store group is released only after the NEXT block's
            # QKV passes — a true data dependency that shifts the RS into
            # the SBUF-only attention window, keeping HBM quiet while the
            # weight slabs for the following block stream in.
            pend = {"att": None, "ffn": None}

            def release_attn_rs(dep_tile):
                bb, held = pend["att"]
                out_ap = d1c[bb][(KD - 4) * P:KD * P, :].rearrange(
                    "(o p) f -> p o f", p=P)
                rel = sb.tile([P, 4, SB], BF16, name="d1t", bufs=2)
                for i in range(4):
                    nc.vector.scalar_tensor_tensor(
                        rel[:, i, :], dep_tile, 0.0, held[:, i, :],
                        mybir.AluOpType.mult, mybir.AluOpType.add)
                nc.sync.dma_start(out=out_ap, in_=rel)
                nc.gpsimd.collective_compute(
                    "ReduceScatter", mybir.AluOpType.add, replica_groups=rg,
                    ins=[d1c[bb][:]], outs=[d1r[bb][:]])
                nc.gpsimd.collective_compute(
                    "AllGather", mybir.AluOpType.bypass, replica_groups=rg,
                    ins=[d1r[bb][:]], outs=[d1g[bb][:]])
                pend["att"] = None

            # =========== attention half ===========
            for b in range(c.NB):
                sl = slice(b * SB, (b + 1) * SB)

                rstd = rmsnorm_stats(xm, "rstd")
                # per-seq-tile [P,1] transposed copies (for v's ACT scale)
                rstdT = sb.tile([P, JT], F32, name="rstdT", bufs=2)
                for st in range(JT):
                    nc.sync.dma_start(out=rstdT[:, st:st + 1],
                                      in_=rstd[0:1, st * P:(st + 1) * P])
                # rope tables pre-scaled by rstd for this block
                cosR = sb.tile([P, SB], BF16, name="cosR", bufs=2)
                nc.sync.dma_start(out=cosR, in_=cosd[:, sl])
                nc.vector.tensor_mul(cosR, cosR, rstd)
                sinR = sb.tile([P, SB], BF16, name="sinR", bufs=2)
                nc.sync.dma_start(out=sinR, in_=sinpm[:, sl])
                nc.vector.tensor_mul(sinR, sinR, rstd)

                qT_blk = sb.tile([P, c.HL, SB], BF16, name="qT_blk", bufs=1)

                def rope_evict(psl, dst, dsl):
                    # psl: 4 psum accumulators; dst[_, h, dsl] targets.
                    # Half-swap via a PE permutation matmul (no DMA on the
                    # critical path — collective traffic can't stall it).
                    for pr in range(FC // 2):
                        raw = sb.tile([P, 2, SB], BF16, name="rp_raw", bufs=2)
                        nc.vector.tensor_copy(raw[:, 0, :], psl[2 * pr])
                        nc.vector.tensor_copy(raw[:, 1, :], psl[2 * pr + 1])
                        sw_ps = [sc_ps(), sc_ps()]
                        nc.tensor.matmul(sw_ps[0], swp_sb, raw[:, 0, :],
                                         start=True, stop=True)
                        nc.tensor.matmul(sw_ps[1], swp_sb, raw[:, 1, :],
                                         start=True, stop=True)
                        cb = cosR[:, None, :].to_broadcast([P, 2, SB])
                        nc.vector.tensor_mul(raw, raw, cb)
                        for i in range(2):
                            swm = sb.tile([P, SB], BF16, name="swm", bufs=2)
                            nc.vector.tensor_mul(swm, sw_ps[i], sinR)
                            nc.vector.tensor_add(dst[:, 2 * pr + i, dsl],
                                                 raw[:, i, :], swm)

                def qkv_pass(w3, n):
                    psl = [acc() for _ in range(FC)]
                    for g in range(0, KD, 4):
                        slab = sb.tile([P, 4, c.FL], BF16, name="wqkv_sl",
                                       bufs=4)
                        nc.scalar.dma_start(out=slab, in_=w3[:, g:g + 4, :])
                        for kk in range(4):
                            for h in range(FC):
                                nc.tensor.matmul(
                                    psl[h], slab[:, kk, h * P:(h + 1) * P],
                                    xm[:, g + kk, :], start=(g + kk == 0),
                                    stop=(g + kk == KD - 1))
                    return psl

                psq = qkv_pass(wq3, "wq_sl")
                rope_evict(psq, qT_blk, slice(0, SB))
                psk = qkv_pass(wk3, "wk_sl")
                rope_evict(psk, kT3, sl)

                # v: seq-major, rstd applied as per-partition ACT scale
                psv = [acc() for _ in range(JT)]
                for g in range(0, KD, 4):
                    slab = sb.tile([P, 4, c.FL], BF16, name="wqkv_sl", bufs=4)
                    nc.scalar.dma_start(out=slab, in_=wv3[:, g:g + 4, :])
                    for kk in range(4):
                        for st in range(JT):
                            nc.tensor.matmul(
                                psv[st], xm[:, g + kk, st * P:(st + 1) * P],
                                slab[:, kk, :], start=(g + kk == 0),
                                stop=(g + kk == KD - 1))
                for st in range(JT):
                    nc.scalar.activation(v3[:, b * JT + st, :], psv[st],
                                         AF.Copy, scale=rstdT[:, st:st + 1])

                # QKV slabs are in: release the previous block's collectives
                if pend["att"] is not None:
                    release_attn_rs(v3[:, b * JT + JT - 1, :])

                # prefetch next block's x while attention runs
                if b + 1 < c.NB:
                    xm_next = load_x_block(b + 1)
                else:
                    xm_next = None

                # -- phase 3: attention, two heads in flight --
                oT_blk = sb.tile([P, c.HL, SB], BF16, name="oT_blk", bufs=1)
                nt = (b + 1) * JT  # valid sk tiles
                for hp in range(c.HL // 2):
                    hs = (2 * hp, 2 * hp + 1)
                    den = {h: acc() for h in hs}
                    o_ps = {h: acc() for h in hs}
                    for t in range(nt):
                        j = t - b * JT
                        for h in hs:
                            scp = sc_ps()
                            nc.tensor.matmul(scp, kT3[:, h, t * P:(t + 1) * P],
                                             qT_blk[:, h, :], start=True,
                                             stop=True)
                            at = sb.tile([P, SB], BF16, name="at", bufs=3)
                            nc.scalar.activation(at, scp, AF.Exp, bias=neg10_b,
                                                 scale=inv_sqrt_dh)
                            if j >= 0:  # diagonal tile: apply causal mask
                                atm = sb.tile([P, SB], BF16, name="atm",
                                              bufs=2)
                                nc.vector.tensor_mul(atm, at, caus_sb[:, j, :])
                                at = atm
                            nc.tensor.matmul(den[h], ones16, at,
                                             start=(t == 0), stop=(t == nt - 1))
                            nc.tensor.matmul(o_ps[h],
                                             v3[:, t, h * P:(h + 1) * P], at,
                                             start=(t == 0), stop=(t == nt - 1))
                    for h in hs:
                        rden = sb.tile([P, SB], F32, name="rden", bufs=1)
                        nc.vector.reciprocal_approx_fast(rden, den[h])
                        nc.vector.tensor_mul(oT_blk[:, h, :], o_ps[h], rden)

                # -- phase 4: out-projection (4 md chunks per group) --
                # Last group's store is held back; releasing it after the
                # next block's QKV passes is what defers the collective.
                for mg in range(0, KD, 4):
                    slab = sb.tile([P, FC, 4 * P], BF16, name="wo_sl", bufs=2)
                    nc.sync.dma_start(out=slab,
                                      in_=wo3[:, :, mg * P:(mg + 4) * P])
                    pso = [acc() for _ in range(4)]
                    for f in range(FC):
                        for i in range(4):
                            nc.tensor.matmul(pso[i],
                                             slab[:, f, i * P:(i + 1) * P],
                                             oT_blk[:, f, :], start=(f == 0),
                                             stop=(f == FC - 1))
                    last = mg == KD - 4
                    d1t = sb.tile([P, 4, SB], BF16,
                                  name="d1hold" if last else "d1t",
                                  bufs=1 if last else 2)
                    for i in range(4):
                        nc.vector.tensor_copy(d1t[:, i, :], pso[i])
                    if last:
                        pend["att"] = (b, d1t)
                    else:
                        nc.sync.dma_start(
                            out=d1c[b][mg * P:(mg + 4) * P, :].rearrange(
                                "(o p) f -> p o f", p=P),
                            in_=d1t)

                xm = xm_next

            # =========== FFN half ===========
            def phase8(b):
                """final residual for this core's d_model slice, block b.
                Emitted one block late so the RS(b) wait is already met."""
                sl = slice(b * SB, (b + 1) * SB)
                d1r3 = d1r[b].rearrange("(o p) f -> p o f", p=P)
                d2r3 = d2r[b].rearrange("(o p) f -> p o f", p=P)
                for g in range(0, c.DS // P, 2):
                    gs_ = slice(g, g + 2)
                    xf = sb.tile([P, 2, SB], F32, name="xf", bufs=1)
                    nc.sync.dma_start(out=xf, in_=xf3[:, gs_, sl])
                    r1 = sb.tile([P, 2, SB], BF16, name="r1", bufs=1)
                    nc.sync.dma_start(out=r1, in_=d1r3[:, gs_, :])
                    r2 = sb.tile([P, 2, SB], BF16, name="r2", bufs=1)
                    nc.sync.dma_start(out=r2, in_=d2r3[:, gs_, :])
                    nc.vector.tensor_add(xf, xf, r1)
                    nc.vector.tensor_add(xf, xf, r2)
                    nc.sync.dma_start(out=out3[:, gs_, sl], in_=xf)

            def load_xmid(b):
                """phase 5: x_mid = x + d1 via gpsimd accumulate-DMA."""
                t = load_x_block(b)
                d1g3 = d1g[b].rearrange("(o p) f -> p o f", p=P)
                for g in range(0, KD, 4):
                    nc.gpsimd.dma_start(out=t[:, g:g + 4, :],
                                        in_=d1g3[:, g:g + 4, :],
                                        accum_op=mybir.AluOpType.add)
                return t

            xm = load_xmid(0)
            for b in range(c.NB):
                sl = slice(b * SB, (b + 1) * SB)
                rstd2 = rmsnorm_stats(xm, "rstd2")

                # -- phase 6: FFN-A: h = (xn@w1) * silu(xn@wg) --
                h_blk = sb.tile([P, DFC, SB], BF16, name="h_blk", bufs=1)
                for j in range(DFC):
                    pa = acc()
                    pg = acc()
                    for g in range(0, KD, 8):
                        slab = sb.tile([P, 8, 2 * P], BF16, name="w1g_sl",
                                       bufs=4)
                        nc.scalar.dma_start(
                            out=slab,
                            in_=w1g3[:, g:g + 8, j * 2 * P:(j + 1) * 2 * P])
                        for kk in range(8):
                            nc.tensor.matmul(pa, slab[:, kk, 0:P],
                                             xm[:, g + kk, :],
                                             start=(g + kk == 0),
                                             stop=(g + kk == KD - 1))
                            nc.tensor.matmul(pg, slab[:, kk, P:2 * P],
                                             xm[:, g + kk, :],
                                             start=(g + kk == 0),
                                             stop=(g + kk == KD - 1))
                    a_t = sb.tile([P, SB], BF16, name="a_t", bufs=2)
                    nc.vector.tensor_mul(a_t, pa, rstd2)
                    g_t = sb.tile([P, SB], BF16, name="g_t", bufs=2)
                    nc.vector.tensor_mul(g_t, pg, rstd2)
                    gs = sb.tile([P, SB], BF16, name="gs", bufs=2)
                    nc.scalar.activation(gs, g_t, AF.Sigmoid, bias=zero_b)
                    nc.vector.tensor_mul(a_t, a_t, g_t)
                    nc.vector.tensor_mul(h_blk[:, j, :], a_t, gs)

                    if j == 2 and pend["att"] is not None:
                        # early FFN-A slabs are in: release pending attn RS
                        release_attn_rs(h_blk[:, 2, :])

                # prefetch next block's x_mid during FFN-B
                xm = load_xmid(b + 1) if b + 1 < c.NB else None

                if b > 0:
                    phase8(b - 1)

                # -- phase 7: FFN-B: d2 = h @ w2 (2 md chunks per group) --
                for mg in range(0, KD, 2):
                    slab = sb.tile([P, DFC, 2 * P], BF16, name="w2_sl", bufs=2)
                    nc.scalar.dma_start(out=slab,
                                        in_=w23[:, :, mg * P:(mg + 2) * P])
                    ps2 = [acc() for _ in range(2)]
                    for j in range(DFC):
                        for i in range(2):
                            nc.tensor.matmul(ps2[i],
                                             slab[:, j, i * P:(i + 1) * P],
                                             h_blk[:, j, :], start=(j == 0),
                                             stop=(j == DFC - 1))
                    d2t = sb.tile([P, 2, SB], BF16, name="d2t", bufs=2)
                    for i in range(2):
                        nc.vector.tensor_copy(d2t[:, i, :], ps2[i])
                    nc.sync.dma_start(
                        out=d2c[b][mg * P:(mg + 2) * P, :].rearrange(
                            "(o p) f -> p o f", p=P),
                        in_=d2t)
                nc.gpsimd.collective_compute(
                    "ReduceScatter", mybir.AluOpType.add, replica_groups=rg,
                    ins=[d2c[b][:]], outs=[d2r[b][:]])

            phase8(c.NB - 1)

    nc.compile()
    return nc


# ---------------- host-side data prep ----------------

def _bf16(a):
    return np.asarray(a, np.float32).astype(ml_dtypes.bfloat16)


def prep_in_maps(c: Cfg, x, wq, bq, wk, bk, wv, bv, wo, bo, scale1, scale2,
                 w1, b1, wg, bg, w2, b2):
    x = np.asarray(x, np.float32).reshape(c.S, c.D)
    for name, bias in (("bq", bq), ("bk", bk), ("bv", bv), ("bo", bo),
                       ("b1", b1), ("bg", bg), ("b2", b2)):
        assert not np.any(np.asarray(bias)), f"{name} must be zero"

    scale1 = np.asarray(scale1, np.float32)
    scale2 = np.asarray(scale2, np.float32)
    wq = np.asarray(wq, np.float32) * scale1[:, None]
    wk = np.asarray(wk, np.float32) * scale1[:, None]
    wv = np.asarray(wv, np.float32) * scale1[:, None]
    wo = np.asarray(wo, np.float32)
    w1 = np.asarray(w1, np.float32) * scale2[:, None]
    wg = np.asarray(wg, np.float32) * scale2[:, None]
    w2 = np.asarray(w2, np.float32)

    # rope permutation within each head's DH columns: [evens | odds]
    perm = np.concatenate([np.arange(0, c.DH, 2), np.arange(1, c.DH, 2)])
    fullperm = np.concatenate([h * c.DH + perm for h in range(c.H)])
    wq_p = wq[:, fullperm]
    wk_p = wk[:, fullperm]

    # pad FFN to DFF_PAD columns/rows
    dff = w1.shape[1]
    pad = c.DFF_PAD - dff
    w1p = np.pad(w1, ((0, 0), (0, pad)))
    wgp = np.pad(wg, ((0, 0), (0, pad)))
    w2p = np.pad(w2, ((0, pad), (0, 0)))

    # rope tables (feat-major, duplicated cos / +-sin halves)
    theta = 1.0 / (10000.0 ** (np.arange(0, c.DH, 2, dtype=np.float32) / c.DH))
    pos = np.arange(c.S, dtype=np.float32)
    ang = pos[None, :] * theta[:, None]            # [DH/2, S]
    cosd = np.concatenate([np.cos(ang), np.cos(ang)], 0).astype(np.float32)
    sinpm = np.concatenate([-np.sin(ang), np.sin(ang)], 0).astype(np.float32)
    assert cosd.shape == (P, c.S)

    # causal masks for diagonal tiles
    caus = np.zeros((c.JT, P, c.SB), np.float32)
    f = np.arange(c.SB)
    p_ = np.arange(P)
    for j in range(c.JT):
        caus[j] = (f[None, :] >= (P * j + p_[:, None])).astype(np.float32)
    caus = caus.astype(ml_dtypes.bfloat16)

    xT = np.ascontiguousarray(x.T)                 # [D, S]
    xT16 = _bf16(xT)
    swp = np.roll(np.eye(P, dtype=np.float32), P // 2, axis=1)

    in_maps = []
    for core in range(c.n_cores):
        fs = slice(core * c.FL, (core + 1) * c.FL)
        dsl = slice(core * c.DFL, (core + 1) * c.DFL)
        ms = slice(core * c.DS, (core + 1) * c.DS)
        w1c = w1p[:, dsl]
        wgc = wgp[:, dsl]
        # interleave per 128-col chunk: [w1 j | wg j] pairs
        w1g = np.empty((c.D, 2 * c.DFL), np.float32)
        for j in range(c.DFC):
            w1g[:, j * 2 * P:j * 2 * P + P] = w1c[:, j * P:(j + 1) * P]
            w1g[:, j * 2 * P + P:(j + 1) * 2 * P] = wgc[:, j * P:(j + 1) * P]
        in_maps.append({
            "xT16": xT16,
            "xTf32s": np.ascontiguousarray(xT[ms]),
            "wq_s": _bf16(wq_p[:, fs]),
            "wk_s": _bf16(wk_p[:, fs]),
            "wv_s": _bf16(wv[:, fs]),
            "wo_s": _bf16(wo[fs, :]),
            "w1g_s": _bf16(w1g),
            "w2_s": _bf16(w2p[dsl, :]),
            "cosd16": _bf16(cosd),
            "sinpm16": _bf16(sinpm),
            "caus": caus,
            "swp": _bf16(swp),
        })
    return in_maps


def assemble_output(c: Cfg, results):
    outT = np.concatenate([results[core]["outT"] for core in range(c.n_cores)],
                          axis=0)               # [D, S]
    return np.ascontiguousarray(outT.T).reshape(1, c.S, c.D)


_CACHED = {}


def kernel(**inputs) -> np.ndarray:
    c = CFG_FULL
    if "nc" not in _CACHED:
        _CACHED["nc"] = build_program(c)
    nc = _CACHED["nc"]
    in_maps = prep_in_maps(c, **{k: np.asarray(inputs[k]) for k in (
        "x", "wq", "bq", "wk", "bk", "wv", "bv", "wo", "bo",
        "scale1", "scale2", "w1", "b1", "wg", "bg", "w2", "b2")})
    res = run_bass_kernel_spmd(nc, in_maps, core_ids=list(range(c.n_cores)))
    return assemble_output(c, res.results)
